# revision 6
# baseline (speedup 1.0000x reference)
"""Trainium2 Bass kernel: EuclideanRadialBasisFunction (squared-distance, GEMM rewrite).

Computes out[b, o] = relu(||x_b||^2 + ||c_o||^2 - 2 * x_b . c_o) for
x: [16384, 1024] fp32, centers: [4096, 1024] fp32 -> out: [16384, 4096] fp32.

Strategy (data-parallel over batch, 8 NeuronCores):
  - shard x along batch: each core computes a [2048, 4096] output tile
  - centers are replicated; the cross term -2*x@c^T runs on the TensorE as a
    K=1024 accumulation (8 chunks of 128) into PSUM, in bf16 (or fp8 DoubleRow)
  - ||x||^2 and ||c||^2 are folded in by the epilogue:
      DVE:  tmp = (psum + x_sq[b]) + c_sq_bcast      (scalar_tensor_tensor)
      ACT:  out = relu(tmp)
  - host pre-computes the (tiny) row norms in fp32 and pre-transposes/casts the
    GEMM operands so the device does pure matmul + 2-op epilogue + DMA.
"""

import os
from contextlib import ExitStack

import numpy as np
import ml_dtypes

B, IN, OUT = 16384, 1024, 4096
NCORES = 8
BS = B // NCORES          # 2048 batch rows per core
NT = BS // 128            # 16 batch tiles of 128 rows
KC = IN // 128            # 8 contraction chunks of 128
NBANK = 512               # matmul free-dim (one PSUM bank, fp32)
HALF = 2048               # output columns per PSUM half (4 banks)

# "bf16" (safe) or "fp8dr" (fp8 e4m3 + DoubleRow, ~1.5x TensorE throughput)
VARIANT = os.environ.get("RBF_VARIANT", "bf16")

_CACHE = {}


def _build_nc(variant):
    import concourse.bacc as bacc
    import concourse.bass as bass
    import concourse.mybir as mybir
    import concourse.tile as tile

    dt = mybir.dt
    wdt = dt.bfloat16 if variant == "bf16" else dt.float8e4

    nc = bacc.Bacc("TRN2", target_bir_lowering=False, debug=False)

    # xt[t, p, k, m] = -2 * x[core_row0 + t*128 + m, k*128 + p]
    xt_d = nc.dram_tensor("xt", [NT, 128, KC, 128], wdt, kind="ExternalInput")
    # ct[p, k, o] = centers[o, k*128 + p]
    ct_d = nc.dram_tensor("ct", [128, KC, OUT], wdt, kind="ExternalInput")
    # csq[p, o] = ||centers[o]||^2  (broadcast along partitions)
    csq_d = nc.dram_tensor("csq", [128, OUT], dt.float32, kind="ExternalInput")
    # xsq[p, t] = ||x[core_row0 + t*128 + p]||^2
    xsq_d = nc.dram_tensor("xsq", [128, NT], dt.float32, kind="ExternalInput")
    out_d = nc.dram_tensor("out", [BS, OUT], dt.float32, kind="ExternalOutput")

    relu = mybir.ActivationFunctionType.Relu
    add = mybir.AluOpType.add

    with tile.TileContext(nc) as tc:
        with ExitStack() as ctx:
            const = ctx.enter_context(tc.tile_pool(name="const", bufs=1))
            xtp = ctx.enter_context(tc.tile_pool(name="xtp", bufs=3))
            psp = ctx.enter_context(tc.tile_pool(name="psp", bufs=2, space="PSUM"))
            tmpp = ctx.enter_context(tc.tile_pool(name="tmpp", bufs=3))
            outp = ctx.enter_context(tc.tile_pool(name="outp", bufs=3))

            ct = const.tile([128, KC, OUT], wdt)
            for k in range(KC):
                nc.sync.dma_start(ct[:, k, :], ct_d.ap()[:, k, :])
            csq = const.tile([128, OUT], dt.float32)
            nc.sync.dma_start(csq[:], csq_d.ap())
            xsq = const.tile([128, NT], dt.float32)
            nc.sync.dma_start(xsq[:], xsq_d.ap())

            for t in range(NT):
                xt = xtp.tile([128, KC, 128], wdt)
                nc.sync.dma_start(xt[:], xt_d.ap()[t])

                for h in range(2):
                    ps = psp.tile([128, HALF], dt.float32)
                    if variant == "bf16":
                        for k in range(KC):
                            lhsT = xt[:, k, :]
                            for nb in range(HALF // NBANK):
                                o0 = h * HALF + nb * NBANK
                                nc.tensor.matmul(
                                    ps[:, bass.ts(nb, NBANK)],
                                    lhsT,
                                    ct[:, k, o0 : o0 + NBANK],
                                    start=(k == 0),
                                    stop=(k == KC - 1),
                                )
                    else:
                        for kp in range(KC // 2):
                            lhsT = xt[:, 2 * kp : 2 * kp + 2, :]
                            for nb in range(HALF // NBANK):
                                o0 = h * HALF + nb * NBANK
                                nc.tensor.matmul(
                                    ps[:, bass.ts(nb, NBANK)],
                                    lhsT,
                                    ct[:, 2 * kp : 2 * kp + 2, o0 : o0 + NBANK],
                                    start=(kp == 0),
                                    stop=(kp == KC // 2 - 1),
                                    perf_mode=mybir.MatmulPerfMode.DoubleRow,
                                )

                    tmp = tmpp.tile([128, HALF], dt.float32)
                    nc.vector.scalar_tensor_tensor(
                        tmp[:],
                        ps[:],
                        xsq[:, t : t + 1],
                        csq[:, h * HALF : (h + 1) * HALF],
                        add,
                        add,
                    )
                    ot = outp.tile([128, HALF], dt.float32)
                    nc.scalar.activation(ot[:], tmp[:], relu)
                    nc.sync.dma_start(
                        out_d.ap()[t * 128 : (t + 1) * 128, h * HALF : (h + 1) * HALF],
                        ot[:],
                    )
    nc.compile()
    return nc


def _get_runner(variant):
    """Compile the Bass program and return a cached SPMD runner.

    Same mechanism run_bass_kernel_spmd uses under axon (bass_exec custom call
    -> PJRT shard_map over the 8 NeuronCores), but with the jitted callable
    cached so repeated calls don't re-trace, and without the donated zero
    output buffers (this kernel writes every output element).
    """
    if variant in _CACHE:
        return _CACHE[variant]

    import jax
    from jax.experimental.shard_map import shard_map
    from jax.sharding import Mesh, PartitionSpec

    import concourse.mybir as mybir
    from concourse.bass2jax import (
        _bass_exec_p,
        install_neuronx_cc_hook,
        partition_id_tensor,
    )

    install_neuronx_cc_hook()
    nc = _build_nc(variant)

    partition_name = nc.partition_id_tensor.name if nc.partition_id_tensor else None
    in_names = []
    out_names = []
    out_avals = []
    for alloc in nc.m.functions[0].allocations:
        if not isinstance(alloc, mybir.MemoryLocationSet):
            continue
        if not alloc.memorylocations:
            continue
        name = alloc.memorylocations[0].name
        if alloc.kind == "ExternalInput":
            if name != partition_name:
                in_names.append(name)
        elif alloc.kind == "ExternalOutput":
            out_names.append(name)
            out_avals.append(
                jax.core.ShapedArray(
                    tuple(alloc.tensor_shape), mybir.dt.np(alloc.dtype)
                )
            )

    bind_names = tuple(in_names) + ((partition_name,) if partition_name else ())

    def _body(*args):
        operands = list(args)
        if partition_name is not None:
            operands.append(partition_id_tensor())
        outs = _bass_exec_p.bind(
            *operands,
            out_avals=tuple(out_avals),
            in_names=bind_names,
            out_names=tuple(out_names),
            lowering_input_output_aliases=(),
            sim_require_finite=True,
            sim_require_nnan=True,
            nc=nc,
        )
        return tuple(outs)

    devices = jax.devices()[:NCORES]
    assert len(devices) == NCORES, f"need {NCORES} cores, got {len(devices)}"
    mesh = Mesh(np.asarray(devices), ("core",))
    sharded = jax.jit(
        shard_map(
            _body,
            mesh=mesh,
            in_specs=(PartitionSpec("core"),) * len(in_names),
            out_specs=(PartitionSpec("core"),) * len(out_names),
            check_rep=False,
        )
    )

    def run(in_maps):
        concat = [
            np.concatenate([np.asarray(m[name]) for m in in_maps], axis=0)
            for name in in_names
        ]
        outs = sharded(*concat)
        return {name: np.asarray(arr) for name, arr in zip(out_names, outs)}

    runner = {
        "run": run,
        "sharded": sharded,
        "in_names": in_names,
        "out_names": out_names,
        "nc": nc,
    }
    _CACHE[variant] = runner
    return runner


def _prepare_in_maps(x, centers, variant):
    x = np.ascontiguousarray(np.asarray(x, dtype=np.float32))
    centers = np.ascontiguousarray(np.asarray(centers, dtype=np.float32))
    assert x.shape == (B, IN) and centers.shape == (OUT, IN)

    np_wdt = ml_dtypes.bfloat16 if variant == "bf16" else ml_dtypes.float8_e4m3

    x_sq = np.einsum("bi,bi->b", x, x, dtype=np.float32)
    c_sq = np.einsum("oi,oi->o", centers, centers, dtype=np.float32)
    csq_b = np.ascontiguousarray(np.broadcast_to(c_sq[None, :], (128, OUT)))

    ct_host = np.ascontiguousarray(
        centers.T.astype(np_wdt).reshape(KC, 128, OUT).transpose(1, 0, 2)
    )

    xm2 = (x * np.float32(-2.0)).astype(np_wdt)

    in_maps = []
    for c in range(NCORES):
        xs = xm2[c * BS : (c + 1) * BS]
        xt_host = np.ascontiguousarray(
            xs.reshape(NT, 128, KC, 128).transpose(0, 3, 2, 1)
        )
        xsq_host = np.ascontiguousarray(x_sq[c * BS : (c + 1) * BS].reshape(NT, 128).T)
        in_maps.append(
            {"xt": xt_host, "ct": ct_host, "csq": csq_b, "xsq": xsq_host}
        )
    return in_maps


def kernel(x, centers):
    variant = VARIANT
    runner = _get_runner(variant)
    in_maps = _prepare_in_maps(x, centers, variant)
    outs = runner["run"](in_maps)
    return np.ascontiguousarray(outs["out"].astype(np.float32, copy=False))


def bench(x, centers, iters=20, variant=None):
    """Time the device execution with inputs pre-staged on the NeuronCores.

    Dispatches `iters` back-to-back executions (async) and blocks at the end;
    returns mean seconds per execution. Host prep / transfers excluded.
    """
    import time

    import jax
    from jax.sharding import NamedSharding, PartitionSpec

    variant = variant or VARIANT
    runner = _get_runner(variant)
    in_maps = _prepare_in_maps(x, centers, variant)

    concat = [
        np.concatenate([np.asarray(m[name]) for m in in_maps], axis=0)
        for name in runner["in_names"]
    ]
    devices = jax.devices()[:NCORES]
    from jax.sharding import Mesh

    mesh = Mesh(np.asarray(devices), ("core",))
    sharding = NamedSharding(mesh, PartitionSpec("core"))
    dev_in = [jax.device_put(a, sharding) for a in concat]

    # warmup (also triggers compile on first use)
    out = runner["sharded"](*dev_in)
    jax.block_until_ready(out)

    t0 = time.perf_counter()
    results = []
    for _ in range(iters):
        results.append(runner["sharded"](*dev_in))
    jax.block_until_ready(results)
    t1 = time.perf_counter()
    return (t1 - t0) / iters
